# revision 23
# baseline (speedup 1.0000x reference)
"""GAT (3-layer, DGL GATConv-style) on 8 Trainium2 NeuronCores — v2.

Self-contained kernel: kernel(**inputs) takes the full unsharded inputs
(features [50000,256] f32, src/dst [800000] i32, per-layer W/al/ar/b),
distributes across 8 cores, and returns the full [50000, 64] output.

Per layer, TWO kernel launches (the launch boundary is the global barrier
between the node-projection phase and the edge phase; the host does pure
data layout between launches):

  launch A ("project", 8-way sharded): core c computes node-table tiles
    [c*49, (c+1)*49) of h@[W | W@alm | W@arm]  ->  slab [128, 49*264] f16
    (per row: 256 feat, 4 el, 4 er).
  host: assembles the full surrogate-ordered table [50176, 384] f16,
    extracts per-core er windows, packs next-layer transposed h.
  launch B ("edges", dst-slab partition): per-edge rows gathered with
    dma_gather (520B payload, int16 indices into lo/hi table halves,
    4 SWDGE queues); t = er[dst] (one-hot fp8 matmul) + el (batched
    identity matmul); ex = max(exp(t), exp(0.2t)); weighted
    scatter-aggregation via PE one-hot matmul into psum per 64-dst
    window; epilogue rst = acc/den + (h+b) (+relu+head-mean on final).

Graph structure (tile schedule, one-hot matrices, gather indices) is
precomputed on the host once and reused for all three layers.
"""

import sys

sys.path.insert(0, "/opt/trn_rl_repo")

import inspect
import textwrap

import numpy as np
import ml_dtypes

import concourse.bacc as bacc
import concourse.bass as bass
import concourse.mybir as mybir
import concourse.tile as tile
from concourse.masks import make_identity

F32 = mybir.dt.float32
F16 = mybir.dt.float16
F8 = mybir.dt.float8e4
BF16 = mybir.dt.bfloat16
I16 = mybir.dt.int16

NPF16 = np.float16
NPBF16 = ml_dtypes.bfloat16
NPF8 = mybir.dt.np(F8)

# --- patch dma_gather: drop the (transpose-only) elem_size%256 assert ---
_src = textwrap.dedent(inspect.getsource(bass.BassGpSimd.dma_gather))
_src = _src.replace("elem_size_bytes > 0 and elem_size_bytes % 256 == 0",
                    "elem_size_bytes > 0")
_src = _src.replace("def dma_gather(", "def _dma_gather_relaxed(", 1)
_ns = dict(bass.__dict__)
exec(compile(_src, "patched_dma_gather", "exec"), _ns)
bass.BassGpSimd.dma_gather_relaxed = _ns["_dma_gather_relaxed"]


class Cfg:
    def __init__(self, N, E, D, H, DH, n_cores, win=64, kblk=16, grp=6,
                 out_heads_mean=False, ohd_fp8=False, batched_el=False):
        self.N = N
        self.E = E
        self.D = D
        self.H = H
        self.DH = DH
        self.C = n_cores
        self.WIN = win      # dst nodes per window (psum group)
        self.KBLK = kblk    # edge-tiles per compute block
        self.GRP = grp      # windows per gather group
        slab = -(-N // n_cores)
        slab = -(-slab // win) * win
        while (slab * n_cores) % 128:
            slab += win
        self.NSLAB = slab
        self.NPAD = slab * n_cores
        self.NW = slab // win
        assert self.NPAD % 128 == 0
        assert self.NSLAB % 128 == 0
        self.NT = self.NPAD // 128
        assert self.NT % n_cores == 0
        self.TPC = self.NT // n_cores        # projection tiles per core
        self.ROW = D + H                     # gather payload elems (feat+el)
        self.AROW = D + 2 * H                # projected row elems (feat+el+er)
        self.RSTRIDE = -(-(self.AROW * 2) // 256) * 128  # table row stride
        self.HALF = min(32768, self.NPAD)
        self.HIBASE = self.NPAD - self.HALF
        self.out_heads_mean = out_heads_mean
        self.ohd_fp8 = ohd_fp8
        self.batched_el = batched_el

    def surr(self, n):
        return (n % 128) * self.NT + n // 128


def plan_edges(cfg, src, dst):
    """Common tile schedule + per-core edge tensors (see v1 docstring)."""
    C, WIN, NW, NSLAB, GRP = cfg.C, cfg.WIN, cfg.NW, cfg.NSLAB, cfg.GRP
    core_of = dst // NSLAB
    dloc = dst % NSLAB
    win_of = dloc // WIN

    deg = np.zeros(cfg.NPAD, dtype=np.int64)
    np.add.at(deg, dst, 1)
    zdeg = deg == 0

    surr_src = cfg.surr(src.astype(np.int64))
    half_of = (surr_src >= cfg.HALF).astype(np.int64)  # 0 = lo, 1 = hi

    cnt = np.zeros((C, NW, 2), dtype=np.int64)
    np.add.at(cnt, (core_of, win_of, half_of), 1)
    zz = np.nonzero(zdeg)[0]
    np.add.at(cnt, (zz // NSLAB, (zz % NSLAB) // WIN, np.zeros(len(zz), np.int64)), 1)

    t_lo = -(-cnt[:, :, 0].max(axis=0) // 128)
    t_hi = -(-cnt[:, :, 1].max(axis=0) // 128)
    t_lo = np.maximum(t_lo, (t_lo + t_hi == 0).astype(np.int64))

    wslots = [[] for _ in range(NW)]
    hslots = {}
    groups = []
    T = 0
    for g in range(-(-NW // GRP)):
        ws = list(range(g * GRP, min((g + 1) * GRP, NW)))
        slots = []
        lo0 = T
        for w in ws:
            hslots[(w, 0)] = list(range(T, T + int(t_lo[w])))
            wslots[w] += hslots[(w, 0)]
            slots += [(w, 0)] * int(t_lo[w])
            T += int(t_lo[w])
        lo1 = T
        for w in ws:
            hslots[(w, 1)] = list(range(T, T + int(t_hi[w])))
            wslots[w] += hslots[(w, 1)]
            slots += [(w, 1)] * int(t_hi[w])
            T += int(t_hi[w])
        hi1 = T
        groups.append(dict(slots=slots, lo=(lo0, lo1), hi=(lo1, hi1)))

    ohd_np = NPF8 if cfg.ohd_fp8 else NPF16
    eidx = np.zeros((C, 128, T * 8), dtype=np.int16)
    ohe = np.zeros((C, 128, T * WIN), dtype=NPBF16)
    ohd = np.zeros((C, 64, T * 128), dtype=ohd_np)

    key = (core_of * NW + win_of) * 2 + half_of
    order = np.lexsort((dst, key))
    s_sorted = src[order]
    d_sorted = dst[order]
    cw = key[order]
    starts = np.searchsorted(cw, np.arange(C * NW * 2))
    ends = np.searchsorted(cw, np.arange(C * NW * 2) + 1)

    wrap_r = np.arange(128) % 16
    wrap_c = np.arange(128) // 16

    for c in range(C):
        for w in range(NW):
            base_d = c * NSLAB + w * WIN
            for half in (0, 1):
                kk = (c * NW + w) * 2 + half
                i0, i1 = starts[kk], ends[kk]
                ss = list(s_sorted[i0:i1])
                dd = list((d_sorted[i0:i1] - base_d))
                if half == 0:
                    for dl in range(WIN):
                        if zdeg[base_d + dl]:
                            ss.append(0)
                            dd.append(dl)
                sl_ids = hslots[(w, half)]
                nslots = len(sl_ids) * 128
                assert len(ss) <= nslots, (c, w, half, len(ss), nslots)
                npad = nslots - len(ss)
                ss += [0] * npad
                dd += [-1] * npad
                ss = np.asarray(ss, dtype=np.int64)
                dd = np.asarray(dd, dtype=np.int64)
                rows = cfg.surr(ss)
                if half == 1:
                    rows = rows - cfg.HIBASE
                    rows = np.where(rows < 0, 0, rows)
                for j, t in enumerate(sl_ids):
                    rr = rows[j * 128:(j + 1) * 128]
                    ddj = dd[j * 128:(j + 1) * 128]
                    eidx[c, wrap_r, t * 8 + wrap_c] = rr.astype(np.int16)
                    p = np.nonzero(ddj >= 0)[0]
                    ohe[c, p, t * WIN + ddj[p]] = NPBF16(1.0)
                    ohd[c, ddj[p], t * 128 + p] = ohd_np(1.0)
    for c in range(C):
        eidx[c] = np.tile(eidx[c, :16], (8, 1))
    return dict(groups=groups, wslots=wslots, T=T, eidx=eidx, ohe=ohe, ohd=ohd)


def pack_hT(cfg, h):
    """h [NPAD, D] -> packed transpose [128, NT*D] (f16), vectorized."""
    NT, D = cfg.NT, cfg.D
    KC = D // 128
    # out[p, t*D + j*128 + q] = h[t*128 + q, j*128 + p]
    hr = np.ascontiguousarray(h.astype(NPF16)).reshape(NT, 128, KC, 128)
    return hr.transpose(3, 0, 2, 1).reshape(128, NT * D)


def make_wx(cfg, W, al, ar):
    H, DH = cfg.H, cfg.DH
    alm = np.zeros((cfg.D, H), dtype=np.float64)
    arm = np.zeros((cfg.D, H), dtype=np.float64)
    for h in range(H):
        alm[h * DH:(h + 1) * DH, h] = al[h]
        arm[h * DH:(h + 1) * DH, h] = ar[h]
    Wx = np.concatenate(
        [W.astype(np.float64), W.astype(np.float64) @ alm,
         W.astype(np.float64) @ arm], axis=1)
    return Wx.astype(NPF16)


def build_project(cfg):
    """Launch A: core computes TPC node tiles of h@Wx -> slab f16."""
    D, AROW, TPC = cfg.D, cfg.AROW, cfg.TPC
    KC = D // 128
    AB = 7
    assert TPC % AB == 0

    nc = bacc.Bacc("TRN2", target_bir_lowering=False, debug=False,
                   enable_asserts=False, num_devices=cfg.C)
    hTq = nc.dram_tensor("hTq", [128, TPC * D], F16, kind="ExternalInput")
    Wx = nc.dram_tensor("Wx", [D, AROW], F16, kind="ExternalInput")
    slab = nc.dram_tensor("slab", [128, TPC * AROW], F16, kind="ExternalOutput")

    with tile.TileContext(nc) as tc:
        with (
            tc.tile_pool(name="const", bufs=1) as cpool,
            tc.tile_pool(name="hblk", bufs=3) as hpool,
            tc.tile_pool(name="rowblk", bufs=3) as rpool,
            tc.tile_pool(name="psA", bufs=4, space="PSUM") as psA,
        ):
            wx0 = cpool.tile([128, AROW], F16, tag="wx0")
            wx1 = cpool.tile([128, AROW], F16, tag="wx1")
            nc.sync.dma_start(out=wx0[:], in_=Wx[0:128, :])
            if KC > 1:
                nc.scalar.dma_start(out=wx1[:], in_=Wx[128:256, :])
            for blk in range(TPC // AB):
                hblk = hpool.tile([128, AB * D], F16)
                nc.sync.dma_start(
                    out=hblk[:], in_=hTq[:, blk * AB * D:(blk + 1) * AB * D])
                rowblk = rpool.tile([128, AB * AROW], F16)
                for j in range(AB):
                    ps = psA.tile([128, AROW], F32)
                    for k in range(KC):
                        nc.tensor.matmul(
                            out=ps[:],
                            lhsT=hblk[:, j * D + k * 128:j * D + (k + 1) * 128],
                            rhs=(wx0 if k == 0 else wx1)[:],
                            start=(k == 0), stop=(k == KC - 1))
                    nc.scalar.activation(
                        out=rowblk[:, j * AROW:j * AROW + D].bitcast(BF16),
                        in_=ps[:, 0:D],
                        func=mybir.ActivationFunctionType.Copy)
                    nc.vector.tensor_copy(
                        out=rowblk[:, j * AROW + D:(j + 1) * AROW],
                        in_=ps[:, D:AROW])
                nc.sync.dma_start(
                    out=slab[:, blk * AB * AROW:(blk + 1) * AB * AROW],
                    in_=rowblk[:])
    nc.compile()
    return nc


def build_edge(cfg, plan, final):
    """Launch B: gather + attention + scatter-aggregate + epilogue."""
    N, D, H, ROW = cfg.NPAD, cfg.D, cfg.H, cfg.ROW
    WIN, KBLK = cfg.WIN, cfg.KBLK
    RST = cfg.RSTRIDE
    T = plan["T"]
    DEN = D + H
    OHD = F8 if cfg.ohd_fp8 else F16
    OUTD = cfg.DH if (cfg.out_heads_mean and final) else D
    OUTT = F32 if (cfg.out_heads_mean and final) else F16

    nc = bacc.Bacc("TRN2", target_bir_lowering=False, debug=False,
                   enable_asserts=False, num_devices=cfg.C, num_swdge_queues=4)

    table = nc.dram_tensor("table", [N, RST], F16, kind="ExternalInput")
    erwin_d = nc.dram_tensor("erwin", [64, cfg.NW * H], F16, kind="ExternalInput")
    hb = nc.dram_tensor("hb", [cfg.NSLAB, D], F16, kind="ExternalInput")
    eidx = nc.dram_tensor("eidx", [128, T * 8], I16, kind="ExternalInput")
    ohe_d = nc.dram_tensor("ohe", [128, T * WIN], BF16, kind="ExternalInput")
    ohd_d = nc.dram_tensor("ohd", [64, T * 128], OHD, kind="ExternalInput")
    out = nc.dram_tensor("out", [cfg.NSLAB, OUTD], OUTT, kind="ExternalOutput")

    with tile.TileContext(nc) as tc:
        with (
            tc.tile_pool(name="const", bufs=1) as cpool,
            tc.tile_pool(name="psT", bufs=2, space="PSUM") as psT,
            tc.tile_pool(name="psB", bufs=cfg.GRP, space="PSUM") as psB,
            tc.tile_pool(name="grow", bufs=3) as gpool,
            tc.tile_pool(name="oh", bufs=4) as opool,
            tc.tile_pool(name="exg", bufs=4) as xpool,
            tc.tile_pool(name="tt", bufs=4) as tpool,
            tc.tile_pool(name="epi", bufs=8) as epool,
            tc.tile_pool(name="fin", bufs=2) as fpool,
        ):
            eidx_t = cpool.tile([128, T * 8], I16, tag="eidx")
            nc.sync.dma_start(out=eidx_t[:], in_=eidx[:, :])
            ident = cpool.tile([128, 128], F16, tag="ident")
            make_identity(nc, ident[:])
            erwin_t = cpool.tile([64, cfg.NW * H], F16, tag="erwin")
            nc.scalar.dma_start(out=erwin_t[:], in_=erwin_d[:, :])
            if cfg.ohd_fp8:
                erw = cpool.tile([64, cfg.NW * H], F8, tag="erw8")
                nc.vector.tensor_copy(out=erw[:], in_=erwin_t[:])
            else:
                erw = erwin_t

            qn = [0]
            hbws = {}
            pend_fin = []

            def emit_fin(stage, g0w, nwg):
                # batched relu + head-mean + store for a whole group
                sall = stage[:, 0:nwg * D]
                nc.vector.tensor_relu(out=sall, in_=sall)
                sv = sall.rearrange("p (w h f) -> p w h f", h=H, f=cfg.DH)
                om = fpool.tile([WIN, cfg.GRP * 2 * cfg.DH], F32, tag="om")
                omv = om[:, 0:nwg * 2 * cfg.DH].rearrange(
                    "p (w i f) -> p w i f", i=2, f=cfg.DH)
                nc.vector.tensor_add(
                    out=omv, in0=sv[:, :, 0:2, :], in1=sv[:, :, 2:4, :])
                of = fpool.tile([WIN, cfg.GRP * cfg.DH], F32, tag="of")
                ofv = of[:, 0:nwg * cfg.DH].rearrange(
                    "p (w f) -> p w f", f=cfg.DH)
                nc.vector.tensor_add(
                    out=ofv, in0=omv[:, :, 0, :], in1=omv[:, :, 1, :])
                nc.vector.tensor_scalar_mul(
                    out=of[:, 0:nwg * cfg.DH], in0=of[:, 0:nwg * cfg.DH],
                    scalar1=1.0 / H)
                nc.sync.dma_start(
                    out=out[g0w * WIN:(g0w + nwg) * WIN, :]
                    .rearrange("(w d) f -> d w f", d=WIN),
                    in_=ofv)

            slot_to_win = {}
            for w in range(cfg.NW):
                for s in plan["wslots"][w]:
                    slot_to_win[s] = w

            for g, grp in enumerate(plan["groups"]):
                s_begin = grp["lo"][0]
                s_end = grp["hi"][1]
                nslot = s_end - s_begin
                grow = gpool.tile([128, nslot * ROW], F16, tag="grow")
                for half, (h0, h1) in (("lo", grp["lo"]), ("hi", grp["hi"])):
                    if h1 == h0:
                        continue
                    ni = (h1 - h0) * 128
                    src_ap = (table[0:cfg.HALF, 0:ROW] if half == "lo"
                              else table[cfg.HIBASE:N, 0:ROW])
                    nc.gpsimd.dma_gather_relaxed(
                        out_ap=grow[:, (h0 - s_begin) * ROW:(h1 - s_begin) * ROW]
                        .rearrange("p (t e) -> p t e", e=ROW),
                        in_ap=src_ap,
                        idxs_ap=eidx_t[:, h0 * 8:h1 * 8],
                        num_idxs=ni, num_idxs_reg=ni,
                        elem_size=ROW, elem_step=RST,
                        single_packet=False, queue_num=qn[0] % 4)
                    qn[0] += 1

                accs = {}
                open_w = {}
                g0w = g * cfg.GRP
                nwg = min((g + 1) * cfg.GRP, cfg.NW) - g0w
                fin_mean = cfg.out_heads_mean and final
                if fin_mean:
                    stage = fpool.tile([WIN, cfg.GRP * D], F16, tag="stage")
                for b0 in range(s_begin, s_end, KBLK):
                    b1 = min(b0 + KBLK, s_end)
                    k = b1 - b0
                    ohe_b = opool.tile([128, KBLK * WIN], BF16, tag="ohe")
                    nc.scalar.dma_start(
                        out=ohe_b[:, 0:k * WIN],
                        in_=ohe_d[:, b0 * WIN:b1 * WIN])
                    ohd_b = opool.tile([64, KBLK * 128], OHD, tag="ohd")
                    nc.sync.dma_start(
                        out=ohd_b[:, 0:k * 128],
                        in_=ohd_d[:, b0 * 128:b1 * 128])
                    pst = psT.tile([128, KBLK * H], F32)
                    for j in range(k):
                        s = b0 + j
                        w = slot_to_win[s]
                        nc.tensor.matmul(
                            out=pst[:, j * H:(j + 1) * H],
                            lhsT=ohd_b[:, j * 128:(j + 1) * 128],
                            rhs=erw[:, w * H:(w + 1) * H],
                            start=True, stop=True, skip_group_check=True)
                    # t = er (psum) + el (gathered), on DVE
                    el_in = (grow[:, (b0 - s_begin) * ROW:(b1 - s_begin) * ROW]
                             .rearrange("p (k c) -> p k c", c=ROW)[:, :, D:DEN])
                    tt = tpool.tile([128, KBLK * H], BF16, tag="tt")
                    nc.vector.tensor_add(
                        out=tt[:, 0:k * H].rearrange("p (k h) -> p k h", h=H),
                        in0=pst[:, 0:k * H].rearrange("p (k h) -> p k h", h=H),
                        in1=el_in)
                    xa = tpool.tile([128, KBLK * H], BF16, tag="xa")
                    xb = tpool.tile([128, KBLK * H], BF16, tag="xb")
                    nc.scalar.activation(
                        out=xa[:, 0:k * H], in_=tt[:, 0:k * H],
                        func=mybir.ActivationFunctionType.Exp)
                    nc.scalar.activation(
                        out=xb[:, 0:k * H], in_=tt[:, 0:k * H],
                        func=mybir.ActivationFunctionType.Exp, scale=0.2)
                    exg = xpool.tile([128, KBLK * DEN], BF16, tag="exg")
                    exg_k = exg[:, 0:k * DEN].rearrange("p (k c) -> p k c", c=DEN)
                    nc.vector.tensor_max(
                        out=exg_k[:, :, D:DEN],
                        in0=xa[:, 0:k * H].rearrange("p (k h) -> p k h", h=H),
                        in1=xb[:, 0:k * H].rearrange("p (k h) -> p k h", h=H))
                    grow_k = (grow[:, (b0 - s_begin) * ROW:(b1 - s_begin) * ROW]
                              .rearrange("p (k c) -> p k c", c=ROW))
                    feat_in = grow_k[:, :, 0:D].bitcast(BF16).rearrange(
                        "p k (h f) -> p k h f", f=cfg.DH)
                    ex_in = (exg_k[:, :, D:DEN]
                             .to_broadcast([128, k, H, cfg.DH]))
                    exg_out = exg_k[:, :, 0:D].rearrange(
                        "p k (h f) -> p k h f", f=cfg.DH)
                    nc.vector.tensor_mul(out=exg_out, in0=feat_in, in1=ex_in)

                    for j in range(k):
                        s = b0 + j
                        w = slot_to_win[s]
                        if w not in accs:
                            acc_w = psB.tile([WIN, DEN], F32, tag="acc")
                            accs[w] = acc_w
                            open_w[w] = 0
                            hbw = epool.tile([WIN, D], F16, tag="hbw")
                            nc.scalar.dma_start(
                                out=hbw[:], in_=hb[w * WIN:(w + 1) * WIN, :])
                            hbws[w] = hbw
                        first = open_w[w] == 0
                        last = s == plan["wslots"][w][-1]
                        open_w[w] += 1
                        nc.tensor.matmul(
                            out=accs[w][:],
                            lhsT=ohe_b[:, j * WIN:(j + 1) * WIN],
                            rhs=exg[:, j * DEN:(j + 1) * DEN],
                            start=first, stop=last, skip_group_check=True)
                        if last:
                            acc = accs.pop(w)
                            rec = epool.tile([WIN, H], F32, tag="rec")
                            nc.vector.reciprocal(out=rec[:], in_=acc[:, D:DEN])
                            if fin_mean:
                                rst = stage[:, (w - g0w) * D:(w - g0w + 1) * D]
                            else:
                                rst_t = epool.tile([WIN, D], F16, tag="rst")
                                rst = rst_t[:]
                            rec_in = rec[:].to_broadcast([WIN, H, cfg.DH])
                            acc_in = acc[:, 0:D].rearrange(
                                "p (h f) -> p h f", f=cfg.DH)
                            rst_out = rst.rearrange(
                                "p (h f) -> p h f", f=cfg.DH)
                            nc.vector.tensor_mul(
                                out=rst_out, in0=acc_in, in1=rec_in)
                            nc.vector.tensor_add(
                                out=rst, in0=rst, in1=hbws.pop(w)[:])
                            if not fin_mean:
                                nc.sync.dma_start(
                                    out=out[w * WIN:(w + 1) * WIN, :], in_=rst)

                if fin_mean:
                    # defer this group's reduction by one group so its DVE
                    # ops don't head-of-line-block the next group's work
                    pend_fin.append((stage, g0w, nwg))
                    if len(pend_fin) > 1:
                        emit_fin(*pend_fin.pop(0))

            for args in pend_fin:
                emit_fin(*args)

    nc.compile()
    return nc


# ---------------------------------------------------------------------------
# kernel() entry point
# ---------------------------------------------------------------------------
_CACHE = {}

_N, _E, _D, _H, _DH = 50000, 800000, 256, 4, 64


def _get_built(src, dst):
    key = "built"
    if key in _CACHE:
        return _CACHE[key]
    cfg_mid = Cfg(_N, _E, _D, _H, _DH, n_cores=8, out_heads_mean=False)
    cfg_fin = Cfg(_N, _E, _D, _H, _DH, n_cores=8, out_heads_mean=True)
    plan = plan_edges(cfg_mid, src.astype(np.int64), dst.astype(np.int64))
    nc_prj = build_project(cfg_mid)
    nc_mid = build_edge(cfg_mid, plan, final=False)
    nc_fin = build_edge(cfg_fin, plan, final=True)
    _CACHE[key] = (cfg_mid, cfg_fin, plan, nc_prj, nc_mid, nc_fin)
    return _CACHE[key]


def _assemble_table(cfg, slabs):
    """slabs [C][128, TPC*AROW] f16 -> table [NPAD, RST] f16 + per-core erwin."""
    table = np.zeros((cfg.NPAD, cfg.RSTRIDE), dtype=NPF16)
    tr = table.reshape(128, cfg.NT, cfg.RSTRIDE)
    for c in range(cfg.C):
        tr[:, c * cfg.TPC:(c + 1) * cfg.TPC, 0:cfg.AROW] = (
            slabs[c].reshape(128, cfg.TPC, cfg.AROW))
    erwins = []
    nodes = np.arange(cfg.NSLAB)
    for c in range(cfg.C):
        nn = c * cfg.NSLAB + nodes
        er = tr[nn % 128, nn // 128, cfg.ROW:cfg.AROW]  # [NSLAB, H]
        erwins.append(np.ascontiguousarray(
            er.reshape(cfg.NW, cfg.WIN, cfg.H).transpose(1, 0, 2)
            .reshape(cfg.WIN, cfg.NW * cfg.H)))
    return table, erwins


def kernel(features, src, dst, W0, al0, ar0, b0, W1, al1, ar1, b1,
           W2, al2, ar2, b2, _collect_exec_ns=None):
    from concourse.bass_utils import run_bass_kernel_spmd

    features = np.asarray(features, dtype=np.float32)
    src = np.asarray(src)
    dst = np.asarray(dst)
    cfg_mid, cfg_fin, plan, nc_prj, nc_mid, nc_fin = _get_built(src, dst)
    cfg = cfg_mid
    trace = _collect_exec_ns is not None

    layers = [
        (np.asarray(W0), np.asarray(al0), np.asarray(ar0), np.asarray(b0)),
        (np.asarray(W1), np.asarray(al1), np.asarray(ar1), np.asarray(b1)),
        (np.asarray(W2), np.asarray(al2), np.asarray(ar2), np.asarray(b2)),
    ]
    h = np.zeros((cfg.NPAD, _D), dtype=np.float32)
    h[:_N] = features
    for li, (W, al, ar, b) in enumerate(layers):
        final = li == 2
        # --- launch A: sharded projection ---
        Wx = make_wx(cfg, W, al, ar)
        hT = pack_hT(cfg, h)
        mapsA = []
        for c in range(cfg.C):
            mapsA.append(dict(
                hTq=np.ascontiguousarray(
                    hT[:, c * cfg.TPC * _D:(c + 1) * cfg.TPC * _D]),
                Wx=Wx))
        resA = run_bass_kernel_spmd(nc_prj, mapsA, list(range(8)), trace=trace)
        if trace:
            _collect_exec_ns.append(resA.exec_time_ns)
        slabs = [resA.results[c]["slab"] for c in range(cfg.C)]

        # --- host: assemble table + er windows ---
        table, erwins = _assemble_table(cfg, slabs)

        # --- launch B: edge phase ---
        nc = nc_fin if final else nc_mid
        mapsB = []
        for c in range(cfg.C):
            sl = slice(c * cfg.NSLAB, (c + 1) * cfg.NSLAB)
            hbc = (h[sl] + b[None, :]).astype(NPF16)
            mapsB.append(dict(table=table, erwin=erwins[c], hb=hbc,
                              eidx=plan["eidx"][c], ohe=plan["ohe"][c],
                              ohd=plan["ohd"][c]))
        resB = run_bass_kernel_spmd(nc, mapsB, list(range(8)), trace=trace)
        if trace:
            _collect_exec_ns.append(resB.exec_time_ns)
        outd = _DH if final else _D
        hn = np.zeros((cfg.NPAD, outd), dtype=np.float32)
        for c in range(cfg.C):
            hn[c * cfg.NSLAB:(c + 1) * cfg.NSLAB] = resB.results[c]["out"]
        hn[cfg.N:] = 0.0
        h = hn
    return h[:_N].astype(np.float32)


# revision 24
# speedup vs baseline: 1.1377x; 1.1377x over previous
"""GAT (3-layer, DGL GATConv-style) on 8 Trainium2 NeuronCores — v2.

Self-contained kernel: kernel(**inputs) takes the full unsharded inputs
(features [50000,256] f32, src/dst [800000] i32, per-layer W/al/ar/b),
distributes across 8 cores, and returns the full [50000, 64] output.

Per layer, TWO kernel launches (the launch boundary is the global barrier
between the node-projection phase and the edge phase; the host does pure
data layout between launches):

  launch A ("project", 8-way sharded): core c computes node-table tiles
    [c*49, (c+1)*49) of h@[W | W@alm | W@arm]  ->  slab [128, 49*264] f16
    (per row: 256 feat, 4 el, 4 er).
  host: assembles the full surrogate-ordered table [50176, 384] f16,
    extracts per-core er windows, packs next-layer transposed h.
  launch B ("edges", dst-slab partition): per-edge rows gathered with
    dma_gather (520B payload, int16 indices into lo/hi table halves,
    4 SWDGE queues); t = er[dst] (one-hot fp8 matmul) + el (batched
    identity matmul); ex = max(exp(t), exp(0.2t)); weighted
    scatter-aggregation via PE one-hot matmul into psum per 64-dst
    window; epilogue rst = acc/den + (h+b) (+relu+head-mean on final).

Graph structure (tile schedule, one-hot matrices, gather indices) is
precomputed on the host once and reused for all three layers.
"""

import sys

sys.path.insert(0, "/opt/trn_rl_repo")

import inspect
import textwrap

import numpy as np
import ml_dtypes

import concourse.bacc as bacc
import concourse.bass as bass
import concourse.mybir as mybir
import concourse.tile as tile
from concourse.masks import make_identity

F32 = mybir.dt.float32
F16 = mybir.dt.float16
F8 = mybir.dt.float8e4
BF16 = mybir.dt.bfloat16
I16 = mybir.dt.int16

NPF16 = np.float16
NPBF16 = ml_dtypes.bfloat16
NPF8 = mybir.dt.np(F8)

# --- patch dma_gather: drop the (transpose-only) elem_size%256 assert ---
_src = textwrap.dedent(inspect.getsource(bass.BassGpSimd.dma_gather))
_src = _src.replace("elem_size_bytes > 0 and elem_size_bytes % 256 == 0",
                    "elem_size_bytes > 0")
_src = _src.replace("def dma_gather(", "def _dma_gather_relaxed(", 1)
_ns = dict(bass.__dict__)
exec(compile(_src, "patched_dma_gather", "exec"), _ns)
bass.BassGpSimd.dma_gather_relaxed = _ns["_dma_gather_relaxed"]


class Cfg:
    def __init__(self, N, E, D, H, DH, n_cores, win=64, kblk=16, grp=6,
                 out_heads_mean=False, ohd_fp8=False, batched_el=False):
        self.N = N
        self.E = E
        self.D = D
        self.H = H
        self.DH = DH
        self.C = n_cores
        self.WIN = win      # dst nodes per window (psum group)
        self.KBLK = kblk    # edge-tiles per compute block
        self.GRP = grp      # windows per gather group
        slab = -(-N // n_cores)
        slab = -(-slab // win) * win
        while (slab * n_cores) % 128:
            slab += win
        self.NSLAB = slab
        self.NPAD = slab * n_cores
        self.NW = slab // win
        assert self.NPAD % 128 == 0
        assert self.NSLAB % 128 == 0
        self.NT = self.NPAD // 128
        assert self.NT % n_cores == 0
        self.TPC = self.NT // n_cores        # projection tiles per core
        self.ROW = D + H                     # gather payload elems (feat+el)
        self.AROW = D + 2 * H                # projected row elems (feat+el+er)
        self.RSTRIDE = -(-(self.AROW * 2) // 256) * 128  # table row stride
        self.HALF = min(32768, self.NPAD)
        self.HIBASE = self.NPAD - self.HALF
        self.out_heads_mean = out_heads_mean
        self.ohd_fp8 = ohd_fp8
        self.batched_el = batched_el

    def surr(self, n):
        return (n % 128) * self.NT + n // 128


def plan_edges(cfg, src, dst):
    """Common tile schedule + per-core edge tensors (see v1 docstring)."""
    C, WIN, NW, NSLAB, GRP = cfg.C, cfg.WIN, cfg.NW, cfg.NSLAB, cfg.GRP
    core_of = dst // NSLAB
    dloc = dst % NSLAB
    win_of = dloc // WIN

    deg = np.zeros(cfg.NPAD, dtype=np.int64)
    np.add.at(deg, dst, 1)
    zdeg = deg == 0

    surr_src = cfg.surr(src.astype(np.int64))
    half_of = (surr_src >= cfg.HALF).astype(np.int64)  # 0 = lo, 1 = hi

    cnt = np.zeros((C, NW, 2), dtype=np.int64)
    np.add.at(cnt, (core_of, win_of, half_of), 1)
    zz = np.nonzero(zdeg)[0]
    np.add.at(cnt, (zz // NSLAB, (zz % NSLAB) // WIN, np.zeros(len(zz), np.int64)), 1)

    t_lo = -(-cnt[:, :, 0].max(axis=0) // 128)
    t_hi = -(-cnt[:, :, 1].max(axis=0) // 128)
    t_lo = np.maximum(t_lo, (t_lo + t_hi == 0).astype(np.int64))

    wslots = [[] for _ in range(NW)]
    hslots = {}
    groups = []
    T = 0
    for g in range(-(-NW // GRP)):
        ws = list(range(g * GRP, min((g + 1) * GRP, NW)))
        slots = []
        lo0 = T
        for w in ws:
            hslots[(w, 0)] = list(range(T, T + int(t_lo[w])))
            wslots[w] += hslots[(w, 0)]
            slots += [(w, 0)] * int(t_lo[w])
            T += int(t_lo[w])
        lo1 = T
        for w in ws:
            hslots[(w, 1)] = list(range(T, T + int(t_hi[w])))
            wslots[w] += hslots[(w, 1)]
            slots += [(w, 1)] * int(t_hi[w])
            T += int(t_hi[w])
        hi1 = T
        groups.append(dict(slots=slots, lo=(lo0, lo1), hi=(lo1, hi1)))

    ohd_np = NPF8 if cfg.ohd_fp8 else NPF16
    eidx = np.zeros((C, 128, T * 8), dtype=np.int16)
    ohe = np.zeros((C, 128, T * WIN), dtype=NPBF16)
    ohd = np.zeros((C, 64, T * 128), dtype=ohd_np)

    key = (core_of * NW + win_of) * 2 + half_of
    order = np.lexsort((dst, key))
    s_sorted = src[order]
    d_sorted = dst[order]
    cw = key[order]
    starts = np.searchsorted(cw, np.arange(C * NW * 2))
    ends = np.searchsorted(cw, np.arange(C * NW * 2) + 1)

    wrap_r = np.arange(128) % 16
    wrap_c = np.arange(128) // 16

    for c in range(C):
        for w in range(NW):
            base_d = c * NSLAB + w * WIN
            for half in (0, 1):
                kk = (c * NW + w) * 2 + half
                i0, i1 = starts[kk], ends[kk]
                ss = list(s_sorted[i0:i1])
                dd = list((d_sorted[i0:i1] - base_d))
                if half == 0:
                    for dl in range(WIN):
                        if zdeg[base_d + dl]:
                            ss.append(0)
                            dd.append(dl)
                sl_ids = hslots[(w, half)]
                nslots = len(sl_ids) * 128
                assert len(ss) <= nslots, (c, w, half, len(ss), nslots)
                npad = nslots - len(ss)
                ss += [0] * npad
                dd += [-1] * npad
                ss = np.asarray(ss, dtype=np.int64)
                dd = np.asarray(dd, dtype=np.int64)
                rows = cfg.surr(ss)
                if half == 1:
                    rows = rows - cfg.HIBASE
                    rows = np.where(rows < 0, 0, rows)
                for j, t in enumerate(sl_ids):
                    rr = rows[j * 128:(j + 1) * 128]
                    ddj = dd[j * 128:(j + 1) * 128]
                    eidx[c, wrap_r, t * 8 + wrap_c] = rr.astype(np.int16)
                    p = np.nonzero(ddj >= 0)[0]
                    ohe[c, p, t * WIN + ddj[p]] = NPBF16(1.0)
                    ohd[c, ddj[p], t * 128 + p] = ohd_np(1.0)
    for c in range(C):
        eidx[c] = np.tile(eidx[c, :16], (8, 1))
    return dict(groups=groups, wslots=wslots, T=T, eidx=eidx, ohe=ohe, ohd=ohd)


def pack_hT(cfg, h):
    """h [NPAD, D] -> packed transpose [128, NT*D] (f16), vectorized."""
    NT, D = cfg.NT, cfg.D
    KC = D // 128
    # out[p, t*D + j*128 + q] = h[t*128 + q, j*128 + p]
    hr = np.ascontiguousarray(h.astype(NPF16)).reshape(NT, 128, KC, 128)
    return hr.transpose(3, 0, 2, 1).reshape(128, NT * D)


def make_wx(cfg, W, al, ar):
    H, DH = cfg.H, cfg.DH
    alm = np.zeros((cfg.D, H), dtype=np.float64)
    arm = np.zeros((cfg.D, H), dtype=np.float64)
    for h in range(H):
        alm[h * DH:(h + 1) * DH, h] = al[h]
        arm[h * DH:(h + 1) * DH, h] = ar[h]
    Wx = np.concatenate(
        [W.astype(np.float64), W.astype(np.float64) @ alm,
         W.astype(np.float64) @ arm], axis=1)
    return Wx.astype(NPF16)


def build_project(cfg):
    """Launch A: core computes TPC node tiles of h@Wx -> slab f16."""
    D, AROW, TPC = cfg.D, cfg.AROW, cfg.TPC
    KC = D // 128
    AB = 7
    assert TPC % AB == 0

    nc = bacc.Bacc("TRN2", target_bir_lowering=False, debug=False,
                   enable_asserts=False, num_devices=cfg.C)
    hTq = nc.dram_tensor("hTq", [128, TPC * D], F16, kind="ExternalInput")
    Wx = nc.dram_tensor("Wx", [D, AROW], F16, kind="ExternalInput")
    slab = nc.dram_tensor("slab", [128, TPC * AROW], F16, kind="ExternalOutput")

    with tile.TileContext(nc) as tc:
        with (
            tc.tile_pool(name="const", bufs=1) as cpool,
            tc.tile_pool(name="hblk", bufs=3) as hpool,
            tc.tile_pool(name="rowblk", bufs=3) as rpool,
            tc.tile_pool(name="psA", bufs=4, space="PSUM") as psA,
        ):
            wx0 = cpool.tile([128, AROW], F16, tag="wx0")
            wx1 = cpool.tile([128, AROW], F16, tag="wx1")
            nc.sync.dma_start(out=wx0[:], in_=Wx[0:128, :])
            if KC > 1:
                nc.scalar.dma_start(out=wx1[:], in_=Wx[128:256, :])
            for blk in range(TPC // AB):
                hblk = hpool.tile([128, AB * D], F16)
                nc.sync.dma_start(
                    out=hblk[:], in_=hTq[:, blk * AB * D:(blk + 1) * AB * D])
                rowblk = rpool.tile([128, AB * AROW], F16)
                for j in range(AB):
                    ps = psA.tile([128, AROW], F32)
                    for k in range(KC):
                        nc.tensor.matmul(
                            out=ps[:],
                            lhsT=hblk[:, j * D + k * 128:j * D + (k + 1) * 128],
                            rhs=(wx0 if k == 0 else wx1)[:],
                            start=(k == 0), stop=(k == KC - 1))
                    nc.scalar.activation(
                        out=rowblk[:, j * AROW:j * AROW + D].bitcast(BF16),
                        in_=ps[:, 0:D],
                        func=mybir.ActivationFunctionType.Copy)
                    nc.vector.tensor_copy(
                        out=rowblk[:, j * AROW + D:(j + 1) * AROW],
                        in_=ps[:, D:AROW])
                nc.sync.dma_start(
                    out=slab[:, blk * AB * AROW:(blk + 1) * AB * AROW],
                    in_=rowblk[:])
    nc.compile()
    return nc


def build_edge(cfg, plan, final):
    """Launch B: gather + attention + scatter-aggregate + epilogue."""
    N, D, H, ROW = cfg.NPAD, cfg.D, cfg.H, cfg.ROW
    WIN, KBLK = cfg.WIN, cfg.KBLK
    RST = cfg.RSTRIDE
    T = plan["T"]
    DEN = D + H
    OHD = F8 if cfg.ohd_fp8 else F16
    OUTD = cfg.DH if (cfg.out_heads_mean and final) else D
    OUTT = F32 if (cfg.out_heads_mean and final) else F16

    nc = bacc.Bacc("TRN2", target_bir_lowering=False, debug=False,
                   enable_asserts=False, num_devices=cfg.C, num_swdge_queues=4)

    table = nc.dram_tensor("table", [N, RST], F16, kind="ExternalInput")
    erwin_d = nc.dram_tensor("erwin", [64, cfg.NW * H], F16, kind="ExternalInput")
    hb = nc.dram_tensor("hb", [cfg.NSLAB, D], F16, kind="ExternalInput")
    eidx = nc.dram_tensor("eidx", [128, T * 8], I16, kind="ExternalInput")
    ohe_d = nc.dram_tensor("ohe", [128, T * WIN], BF16, kind="ExternalInput")
    ohd_d = nc.dram_tensor("ohd", [64, T * 128], OHD, kind="ExternalInput")
    out = nc.dram_tensor("out", [cfg.NSLAB, OUTD], OUTT, kind="ExternalOutput")

    with tile.TileContext(nc) as tc:
        with (
            tc.tile_pool(name="const", bufs=1) as cpool,
            tc.tile_pool(name="psT", bufs=2, space="PSUM") as psT,
            tc.tile_pool(name="psB", bufs=cfg.GRP, space="PSUM") as psB,
            tc.tile_pool(name="grow", bufs=3) as gpool,
            tc.tile_pool(name="oh", bufs=4) as opool,
            tc.tile_pool(name="exg", bufs=4) as xpool,
            tc.tile_pool(name="tt", bufs=4) as tpool,
            tc.tile_pool(name="epi", bufs=8) as epool,
            tc.tile_pool(name="fin", bufs=2) as fpool,
        ):
            eidx_t = cpool.tile([128, T * 8], I16, tag="eidx")
            nc.sync.dma_start(out=eidx_t[:], in_=eidx[:, :])
            ident = cpool.tile([128, 128], F16, tag="ident")
            make_identity(nc, ident[:])
            erwin_t = cpool.tile([64, cfg.NW * H], F16, tag="erwin")
            nc.scalar.dma_start(out=erwin_t[:], in_=erwin_d[:, :])
            if cfg.ohd_fp8:
                erw = cpool.tile([64, cfg.NW * H], F8, tag="erw8")
                nc.vector.tensor_copy(out=erw[:], in_=erwin_t[:])
            else:
                erw = erwin_t

            qn = [0]
            hbws = {}
            pend_fin = []

            def emit_fin(stage, g0w, nwg):
                # batched relu + head-mean + store for a whole group
                sall = stage[:, 0:nwg * D]
                nc.scalar.activation(
                    out=sall, in_=sall,
                    func=mybir.ActivationFunctionType.Relu)
                sv = sall.rearrange("p (w h f) -> p w h f", h=H, f=cfg.DH)
                om = fpool.tile([WIN, cfg.GRP * 2 * cfg.DH], F32, tag="om")
                omv = om[:, 0:nwg * 2 * cfg.DH].rearrange(
                    "p (w i f) -> p w i f", i=2, f=cfg.DH)
                nc.vector.tensor_add(
                    out=omv, in0=sv[:, :, 0:2, :], in1=sv[:, :, 2:4, :])
                of = fpool.tile([WIN, cfg.GRP * cfg.DH], F32, tag="of")
                ofv = of[:, 0:nwg * cfg.DH].rearrange(
                    "p (w f) -> p w f", f=cfg.DH)
                nc.vector.tensor_add(
                    out=ofv, in0=omv[:, :, 0, :], in1=omv[:, :, 1, :])
                nc.scalar.activation(
                    out=of[:, 0:nwg * cfg.DH], in_=of[:, 0:nwg * cfg.DH],
                    func=mybir.ActivationFunctionType.Copy, scale=1.0 / H)
                nc.sync.dma_start(
                    out=out[g0w * WIN:(g0w + nwg) * WIN, :]
                    .rearrange("(w d) f -> d w f", d=WIN),
                    in_=ofv)

            slot_to_win = {}
            for w in range(cfg.NW):
                for s in plan["wslots"][w]:
                    slot_to_win[s] = w

            for g, grp in enumerate(plan["groups"]):
                s_begin = grp["lo"][0]
                s_end = grp["hi"][1]
                nslot = s_end - s_begin
                grow = gpool.tile([128, nslot * ROW], F16, tag="grow")
                for half, (h0, h1) in (("lo", grp["lo"]), ("hi", grp["hi"])):
                    if h1 == h0:
                        continue
                    ni = (h1 - h0) * 128
                    src_ap = (table[0:cfg.HALF, 0:ROW] if half == "lo"
                              else table[cfg.HIBASE:N, 0:ROW])
                    nc.gpsimd.dma_gather_relaxed(
                        out_ap=grow[:, (h0 - s_begin) * ROW:(h1 - s_begin) * ROW]
                        .rearrange("p (t e) -> p t e", e=ROW),
                        in_ap=src_ap,
                        idxs_ap=eidx_t[:, h0 * 8:h1 * 8],
                        num_idxs=ni, num_idxs_reg=ni,
                        elem_size=ROW, elem_step=RST,
                        single_packet=False, queue_num=qn[0] % 4)
                    qn[0] += 1

                accs = {}
                open_w = {}
                g0w = g * cfg.GRP
                nwg = min((g + 1) * cfg.GRP, cfg.NW) - g0w
                fin_mean = cfg.out_heads_mean and final
                if fin_mean:
                    stage = fpool.tile([WIN, cfg.GRP * D], F16, tag="stage")
                for b0 in range(s_begin, s_end, KBLK):
                    b1 = min(b0 + KBLK, s_end)
                    k = b1 - b0
                    ohe_b = opool.tile([128, KBLK * WIN], BF16, tag="ohe")
                    nc.scalar.dma_start(
                        out=ohe_b[:, 0:k * WIN],
                        in_=ohe_d[:, b0 * WIN:b1 * WIN])
                    ohd_b = opool.tile([64, KBLK * 128], OHD, tag="ohd")
                    nc.sync.dma_start(
                        out=ohd_b[:, 0:k * 128],
                        in_=ohd_d[:, b0 * 128:b1 * 128])
                    pst = psT.tile([128, KBLK * H], F32)
                    for j in range(k):
                        s = b0 + j
                        w = slot_to_win[s]
                        nc.tensor.matmul(
                            out=pst[:, j * H:(j + 1) * H],
                            lhsT=ohd_b[:, j * 128:(j + 1) * 128],
                            rhs=erw[:, w * H:(w + 1) * H],
                            start=True, stop=True, skip_group_check=True)
                    # t = er (psum) + el (gathered), on DVE
                    el_in = (grow[:, (b0 - s_begin) * ROW:(b1 - s_begin) * ROW]
                             .rearrange("p (k c) -> p k c", c=ROW)[:, :, D:DEN])
                    tt = tpool.tile([128, KBLK * H], BF16, tag="tt")
                    nc.vector.tensor_add(
                        out=tt[:, 0:k * H].rearrange("p (k h) -> p k h", h=H),
                        in0=pst[:, 0:k * H].rearrange("p (k h) -> p k h", h=H),
                        in1=el_in)
                    xa = tpool.tile([128, KBLK * H], BF16, tag="xa")
                    xb = tpool.tile([128, KBLK * H], BF16, tag="xb")
                    nc.scalar.activation(
                        out=xa[:, 0:k * H], in_=tt[:, 0:k * H],
                        func=mybir.ActivationFunctionType.Exp)
                    nc.scalar.activation(
                        out=xb[:, 0:k * H], in_=tt[:, 0:k * H],
                        func=mybir.ActivationFunctionType.Exp, scale=0.2)
                    exg = xpool.tile([128, KBLK * DEN], BF16, tag="exg")
                    exg_k = exg[:, 0:k * DEN].rearrange("p (k c) -> p k c", c=DEN)
                    nc.vector.tensor_max(
                        out=exg_k[:, :, D:DEN],
                        in0=xa[:, 0:k * H].rearrange("p (k h) -> p k h", h=H),
                        in1=xb[:, 0:k * H].rearrange("p (k h) -> p k h", h=H))
                    grow_k = (grow[:, (b0 - s_begin) * ROW:(b1 - s_begin) * ROW]
                              .rearrange("p (k c) -> p k c", c=ROW))
                    feat_in = grow_k[:, :, 0:D].bitcast(BF16).rearrange(
                        "p k (h f) -> p k h f", f=cfg.DH)
                    ex_in = (exg_k[:, :, D:DEN]
                             .to_broadcast([128, k, H, cfg.DH]))
                    exg_out = exg_k[:, :, 0:D].rearrange(
                        "p k (h f) -> p k h f", f=cfg.DH)
                    nc.vector.tensor_mul(out=exg_out, in0=feat_in, in1=ex_in)

                    for j in range(k):
                        s = b0 + j
                        w = slot_to_win[s]
                        if w not in accs:
                            acc_w = psB.tile([WIN, DEN], F32, tag="acc")
                            accs[w] = acc_w
                            open_w[w] = 0
                            hbw = epool.tile([WIN, D], F16, tag="hbw")
                            nc.scalar.dma_start(
                                out=hbw[:], in_=hb[w * WIN:(w + 1) * WIN, :])
                            hbws[w] = hbw
                        first = open_w[w] == 0
                        last = s == plan["wslots"][w][-1]
                        open_w[w] += 1
                        nc.tensor.matmul(
                            out=accs[w][:],
                            lhsT=ohe_b[:, j * WIN:(j + 1) * WIN],
                            rhs=exg[:, j * DEN:(j + 1) * DEN],
                            start=first, stop=last, skip_group_check=True)
                        if last:
                            acc = accs.pop(w)
                            rec = epool.tile([WIN, H], F32, tag="rec")
                            nc.vector.reciprocal(out=rec[:], in_=acc[:, D:DEN])
                            if fin_mean:
                                rst = stage[:, (w - g0w) * D:(w - g0w + 1) * D]
                            else:
                                rst_t = epool.tile([WIN, D], F16, tag="rst")
                                rst = rst_t[:]
                            rec_in = rec[:].to_broadcast([WIN, H, cfg.DH])
                            acc_in = acc[:, 0:D].rearrange(
                                "p (h f) -> p h f", f=cfg.DH)
                            rst_out = rst.rearrange(
                                "p (h f) -> p h f", f=cfg.DH)
                            nc.vector.tensor_mul(
                                out=rst_out, in0=acc_in, in1=rec_in)
                            nc.vector.tensor_add(
                                out=rst, in0=rst, in1=hbws.pop(w)[:])
                            if not fin_mean:
                                nc.sync.dma_start(
                                    out=out[w * WIN:(w + 1) * WIN, :], in_=rst)

                if fin_mean:
                    # defer this group's reduction by one group so its DVE
                    # ops don't head-of-line-block the next group's work
                    pend_fin.append((stage, g0w, nwg))
                    if len(pend_fin) > 1:
                        emit_fin(*pend_fin.pop(0))

            for args in pend_fin:
                emit_fin(*args)

    nc.compile()
    return nc


# ---------------------------------------------------------------------------
# kernel() entry point
# ---------------------------------------------------------------------------
_CACHE = {}

_N, _E, _D, _H, _DH = 50000, 800000, 256, 4, 64


def _get_built(src, dst):
    key = "built"
    if key in _CACHE:
        return _CACHE[key]
    cfg_mid = Cfg(_N, _E, _D, _H, _DH, n_cores=8, out_heads_mean=False)
    cfg_fin = Cfg(_N, _E, _D, _H, _DH, n_cores=8, out_heads_mean=True)
    plan = plan_edges(cfg_mid, src.astype(np.int64), dst.astype(np.int64))
    nc_prj = build_project(cfg_mid)
    nc_mid = build_edge(cfg_mid, plan, final=False)
    nc_fin = build_edge(cfg_fin, plan, final=True)
    _CACHE[key] = (cfg_mid, cfg_fin, plan, nc_prj, nc_mid, nc_fin)
    return _CACHE[key]


def _assemble_table(cfg, slabs):
    """slabs [C][128, TPC*AROW] f16 -> table [NPAD, RST] f16 + per-core erwin."""
    table = np.zeros((cfg.NPAD, cfg.RSTRIDE), dtype=NPF16)
    tr = table.reshape(128, cfg.NT, cfg.RSTRIDE)
    for c in range(cfg.C):
        tr[:, c * cfg.TPC:(c + 1) * cfg.TPC, 0:cfg.AROW] = (
            slabs[c].reshape(128, cfg.TPC, cfg.AROW))
    erwins = []
    nodes = np.arange(cfg.NSLAB)
    for c in range(cfg.C):
        nn = c * cfg.NSLAB + nodes
        er = tr[nn % 128, nn // 128, cfg.ROW:cfg.AROW]  # [NSLAB, H]
        erwins.append(np.ascontiguousarray(
            er.reshape(cfg.NW, cfg.WIN, cfg.H).transpose(1, 0, 2)
            .reshape(cfg.WIN, cfg.NW * cfg.H)))
    return table, erwins


def kernel(features, src, dst, W0, al0, ar0, b0, W1, al1, ar1, b1,
           W2, al2, ar2, b2, _collect_exec_ns=None):
    from concourse.bass_utils import run_bass_kernel_spmd

    features = np.asarray(features, dtype=np.float32)
    src = np.asarray(src)
    dst = np.asarray(dst)
    cfg_mid, cfg_fin, plan, nc_prj, nc_mid, nc_fin = _get_built(src, dst)
    cfg = cfg_mid
    trace = _collect_exec_ns is not None

    layers = [
        (np.asarray(W0), np.asarray(al0), np.asarray(ar0), np.asarray(b0)),
        (np.asarray(W1), np.asarray(al1), np.asarray(ar1), np.asarray(b1)),
        (np.asarray(W2), np.asarray(al2), np.asarray(ar2), np.asarray(b2)),
    ]
    h = np.zeros((cfg.NPAD, _D), dtype=np.float32)
    h[:_N] = features
    for li, (W, al, ar, b) in enumerate(layers):
        final = li == 2
        # --- launch A: sharded projection ---
        Wx = make_wx(cfg, W, al, ar)
        hT = pack_hT(cfg, h)
        mapsA = []
        for c in range(cfg.C):
            mapsA.append(dict(
                hTq=np.ascontiguousarray(
                    hT[:, c * cfg.TPC * _D:(c + 1) * cfg.TPC * _D]),
                Wx=Wx))
        resA = run_bass_kernel_spmd(nc_prj, mapsA, list(range(8)), trace=trace)
        if trace:
            _collect_exec_ns.append(resA.exec_time_ns)
        slabs = [resA.results[c]["slab"] for c in range(cfg.C)]

        # --- host: assemble table + er windows ---
        table, erwins = _assemble_table(cfg, slabs)

        # --- launch B: edge phase ---
        nc = nc_fin if final else nc_mid
        mapsB = []
        for c in range(cfg.C):
            sl = slice(c * cfg.NSLAB, (c + 1) * cfg.NSLAB)
            hbc = (h[sl] + b[None, :]).astype(NPF16)
            mapsB.append(dict(table=table, erwin=erwins[c], hb=hbc,
                              eidx=plan["eidx"][c], ohe=plan["ohe"][c],
                              ohd=plan["ohd"][c]))
        resB = run_bass_kernel_spmd(nc, mapsB, list(range(8)), trace=trace)
        if trace:
            _collect_exec_ns.append(resB.exec_time_ns)
        outd = _DH if final else _D
        hn = np.zeros((cfg.NPAD, outd), dtype=np.float32)
        for c in range(cfg.C):
            hn[c * cfg.NSLAB:(c + 1) * cfg.NSLAB] = resB.results[c]["out"]
        hn[cfg.N:] = 0.0
        h = hn
    return h[:_N].astype(np.float32)
